# revision 8
# baseline (speedup 1.0000x reference)
"""TRN2 Bass/Tile kernel for nn_MHA_45964740002076 (v2: collective design).

MHA: x[1,4096,768] -> qkv proj -> 12-head attention (softmax scaled by
1/sqrt(768) AFTER softmax, per reference) -> out proj.

The end-to-end cost is dominated by host prep + H2D volume, so v2 ships
every input exactly once as contiguous zero-copy shards and reassembles
on-device with AllGather collectives:

  per-core inputs (all fp32 views, no host copies):
    xo     [512, 768]   x rows  [512c, 512c+512)     (stacked = x)
    wqkv_s [96, 2304]   Wqkv rows [96c, 96c+96)      (stacked = Wqkv)
    wo_s   [96, 768]    Wo   rows [96c, 96c+96)      (stacked = Wo)
    biasp  [128, 18]    packed head-major bq|bk|bv/sqrt(D) tiles (tiny)
    bo_row [1, 768]     bo (tiny)
  output:
    out    [512, 768]   rows of the final output     (stacked = result)

On-device pipeline (all matmuls bf16 with fp32 PSUM):
  1. cast W shards to bf16, one AllGather -> full Wqkv|Wo [768, 3072];
     extract head-major Wq/Wk/Wv (stride-3 DVE copies) + Wo tiles.
     Meanwhile: load own x, cast bf16, PE-transpose to xoT [128,6,512].
  2. own-slice projections: QT (stays local), KT_own [128,6,512],
     V_own [128,4,12,65] (aug ones column -> softmax denominator).
  3. per head-group g in {0,1,2} (4 heads = 2 pairs each): bounce
     kt/v group slice to DRAM, AllGather -> kvg[8,128,2064]; SBUF-load
     KT full rows + V tiles; attention for the group's 2 pairs
     (exp on ACT engine, no max-sub: |energy| < ~30; denominator via
     augmented V; recip*1/sqrt(D) broadcast by tiny PE matmul).
     Group g+1's gather overlaps group g's attention.
  4. o-proj in natural layout: out[q,o] = attnT^T @ Wo + ones^T @ bo.
"""

import os
import numpy as np

os.environ.setdefault("MYCRO_LOCAL_CACHE", "1")

D = 768
H = 12
DH = 64
N = 4096
NCORES = 8
NLOC = N // NCORES          # 512 rows per core
WSH = D // NCORES           # 96 weight rows per core
PAIRS = H // 2              # 6
GROUPS = 3                  # 2 pairs (4 heads) per group
ITILES = D // 128           # 6
QTILES = NLOC // 128        # 4
JT = NLOC // 128            # own l-tiles (4)
LTILES = N // 128           # 32
KVCOLS = 2 * NLOC + JT * 4 * (DH + 1)   # 1024 + 1040 = 2064
INV_SQRT_D = 1.0 / float(np.sqrt(np.float32(D)))

_cache = {}


def _build_program():
    import concourse.bass as bass
    import concourse.mybir as mybir
    import concourse.tile as tile
    from concourse import bacc
    from concourse.masks import make_identity

    f32 = mybir.dt.float32
    bf16 = mybir.dt.bfloat16
    mult = mybir.AluOpType.mult
    bypass = mybir.AluOpType.bypass
    RG = [list(range(NCORES))]

    nc = bacc.Bacc("TRN2", target_bir_lowering=False, debug=False,
                   num_devices=NCORES)

    xo = nc.dram_tensor("xo", [NLOC, D], f32, kind="ExternalInput").ap()
    wqkv_s = nc.dram_tensor("wqkv_s", [WSH, 3 * D], f32,
                            kind="ExternalInput").ap()
    wo_s = nc.dram_tensor("wo_s", [WSH, D], f32, kind="ExternalInput").ap()
    biasp = nc.dram_tensor("biasp", [128, 18], f32, kind="ExternalInput").ap()
    bo_row = nc.dram_tensor("bo_row", [1, D], f32, kind="ExternalInput").ap()
    out = nc.dram_tensor("out", [NLOC, D], f32, kind="ExternalOutput").ap()

    with tile.TileContext(nc) as tc:
        with (
            tc.tile_pool(name="persist", bufs=1) as persist,
            tc.tile_pool(name="scratch", bufs=1) as scratch,
            tc.tile_pool(name="wrawp", bufs=2) as wrawp,
            tc.tile_pool(name="expp", bufs=3) as expp,
            tc.tile_pool(name="small", bufs=2) as small,
            tc.tile_pool(name="kvsb", bufs=2) as kvsb,
            tc.tile_pool(name="dram", bufs=1, space="DRAM") as dram,
        ):
            # ---- tiny persistent state ----
            bias_t = persist.tile([128, 18], f32, tag="biasp")
            nc.sync.dma_start(bias_t[:], biasp)
            bo_f = persist.tile([1, D], f32, tag="bo_f")
            nc.sync.dma_start(bo_f[:], bo_row)
            bo_bf = persist.tile([1, D], bf16, tag="bo_bf")
            nc.vector.tensor_copy(bo_bf[:], bo_f[:])

            ident = persist.tile([128, 128], bf16, tag="ident")
            make_identity(nc, ident[:])
            ones_row = persist.tile([1, 64], bf16, tag="ones")
            nc.vector.memset(ones_row[:], 1.0)
            ones_q = persist.tile([1, 128], bf16, tag="onesq")
            nc.vector.memset(ones_q[:], 1.0)
            zbias = persist.tile([128, 1], f32, tag="zbias")
            nc.vector.memset(zbias[:], 0.0)

            # ---- W shard: cast + single AllGather (Wqkv || Wo) ----
            wsh_f = scratch.tile([WSH, 4 * D], f32, tag="wsh_f")
            nc.sync.dma_start(wsh_f[:, 0:3 * D], wqkv_s)
            nc.sync.dma_start(wsh_f[:, 3 * D:4 * D], wo_s)
            wsh_bf = scratch.tile([WSH, 4 * D], bf16, tag="wsh_bf")
            nc.vector.tensor_copy(wsh_bf[:], wsh_f[:])
            w_in = dram.tile([WSH, 4 * D], bf16, tag="w_in")
            nc.sync.dma_start(w_in[:], wsh_bf[:])
            wg = dram.tile([D, 4 * D], bf16, tag="wg", addr_space="Shared")
            nc.gpsimd.collective_compute(
                "AllGather", bypass, replica_groups=RG,
                ins=[w_in[:].opt()], outs=[wg[:].opt()],
            )

            # ---- own x: load, cast, PE-transpose to xoT ----
            xoT = persist.tile([128, ITILES, NLOC], bf16, tag="xoT")
            with (
                tc.tile_pool(name="xload", bufs=1) as xload,
                tc.tile_pool(name="tr_ps", bufs=4,
                             space=bass.MemorySpace.PSUM) as tr_ps,
            ):
                xq_f = xload.tile([128, QTILES, D], f32, tag="xq_f")
                nc.sync.dma_start(
                    xq_f[:], xo.rearrange("(qt p) f -> p qt f", p=128)
                )
                xq_bf = xload.tile([128, QTILES, D], bf16, tag="xq_bf")
                nc.vector.tensor_copy(xq_bf[:], xq_f[:])
                for qt in range(QTILES):
                    for ft in range(ITILES):
                        ps = tr_ps.tile([128, 128], bf16, tag="tr")
                        nc.tensor.transpose(
                            ps[:], xq_bf[:, qt, ft * 128:(ft + 1) * 128],
                            ident[:],
                        )
                        nc.vector.tensor_copy(
                            xoT[:, ft, qt * 128:(qt + 1) * 128], ps[:]
                        )

            # ---- W tiles from gathered buffer ----
            wq_t, wk_t, wv_t, wo_t = [], [], [], []
            for it in range(ITILES):
                wraw = wrawp.tile([128, 3 * D], bf16, tag="wraw")
                nc.sync.dma_start(
                    wraw[:], wg[it * 128:(it + 1) * 128, 0:3 * D]
                )
                w3 = wraw[:].rearrange("p (x three) -> p three x", three=3)
                for lst, j in ((wq_t, 0), (wk_t, 1), (wv_t, 2)):
                    t = persist.tile([128, D], bf16, tag=f"w{j}_{it}",
                                     name=f"w{j}_{it}")
                    nc.vector.tensor_copy(t[:], w3[:, j, :])
                    lst.append(t)
                t = persist.tile([128, D], bf16, tag=f"wo_{it}",
                                 name=f"wo_{it}")
                nc.sync.dma_start(t[:], wg[it * 128:(it + 1) * 128,
                                           3 * D:4 * D])
                wo_t.append(t)

            # ---- own-slice projections ----
            qt_t = persist.tile([128, PAIRS, NLOC], bf16, tag="qt")
            kt_own = persist.tile([128, PAIRS, NLOC], bf16, tag="kt_own")
            v_own = persist.tile([128, JT, H, DH + 1], bf16, tag="v_own")
            nc.vector.memset(v_own[:, :, :, DH:DH + 1], 1.0)

            with tc.tile_pool(name="gp_ps", bufs=2,
                              space=bass.MemorySpace.PSUM) as gp_ps:
                for p in range(PAIRS):
                    ps = gp_ps.tile([128, NLOC], f32, tag="gp")
                    for it in range(ITILES):
                        nc.tensor.matmul(
                            ps[:],
                            wq_t[it][:, p * 128:(p + 1) * 128],
                            xoT[:, it, :],
                            start=(it == 0),
                            stop=(it == ITILES - 1),
                        )
                    nc.vector.tensor_scalar_add(
                        qt_t[:, p, :], ps[:], bias_t[:, p:p + 1]
                    )
                for p in range(PAIRS):
                    ps = gp_ps.tile([128, NLOC], f32, tag="gp")
                    for it in range(ITILES):
                        nc.tensor.matmul(
                            ps[:],
                            wk_t[it][:, p * 128:(p + 1) * 128],
                            xoT[:, it, :],
                            start=(it == 0),
                            stop=(it == ITILES - 1),
                        )
                    nc.vector.tensor_scalar_add(
                        kt_own[:, p, :], ps[:], bias_t[:, 6 + p:7 + p]
                    )
                for j in range(JT):
                    for g in range(GROUPS):
                        ps = gp_ps.tile([128, 256], f32, tag="gpv")
                        for it in range(ITILES):
                            nc.tensor.matmul(
                                ps[:],
                                xoT[:, it, j * 128:(j + 1) * 128],
                                wv_t[it][:, g * 256:(g + 1) * 256],
                                start=(it == 0),
                                stop=(it == ITILES - 1),
                            )
                        nc.vector.tensor_copy(
                            v_own[:, j, 4 * g:4 * g + 4, 0:DH],
                            ps[:].rearrange("p (h v) -> p h v", v=DH),
                        )

            # ---- chunked KT/V AllGather (one per head-group) ----
            kvg = []
            for g in range(GROUPS):
                kv_in = dram.tile([128, KVCOLS], bf16, tag=f"kv_in{g}",
                                  name=f"kv_in{g}")
                nc.sync.dma_start(
                    kv_in[:, 0:2 * NLOC],
                    kt_own[:, 2 * g:2 * g + 2, :],
                )
                nc.sync.dma_start(
                    kv_in[:, 2 * NLOC:KVCOLS].rearrange(
                        "p (j h c) -> p j h c", j=JT, h=4
                    ),
                    v_own[:, :, 4 * g:4 * g + 4, :],
                )
                kvo = dram.tile([NCORES, 128, KVCOLS], bf16, tag=f"kvg{g}",
                                name=f"kvg{g}", addr_space="Shared")
                nc.gpsimd.collective_compute(
                    "AllGather", bypass, replica_groups=RG,
                    ins=[kv_in[:].opt()], outs=[kvo[:].opt()],
                )
                kvg.append(kvo)

            # ---- attention ----
            attn_t = [
                persist.tile([128, NLOC], bf16, tag=f"attn{p}",
                             name=f"attn{p}")
                for p in range(PAIRS)
            ]

            with (
                tc.tile_pool(name="sc_ps", bufs=2,
                             space=bass.MemorySpace.PSUM) as sc_ps,
                tc.tile_pool(name="acc_ps", bufs=2,
                             space=bass.MemorySpace.PSUM) as acc_ps,
            ):
                for g in range(GROUPS):
                    gpairs = (2 * g, 2 * g + 1)
                    kt_g = kvsb.tile([128, 2, N], bf16, tag="ktg",
                                     name=f"ktg{g}")
                    v_g = kvsb.tile([128, LTILES, 4, DH + 1], bf16,
                                    tag="vg", name=f"vg{g}")
                    for i in range(2):
                        nc.sync.dma_start(
                            kt_g[:, i, :].rearrange("part (r l) -> part r l",
                                                    r=NCORES),
                            kvg[g][:, :, i * NLOC:(i + 1) * NLOC].rearrange(
                                "r part l -> part r l"),
                        )
                    nc.sync.dma_start(
                        v_g[:].rearrange(
                            "part (r j) h c -> part r j h c", r=NCORES
                        ),
                        kvg[g][:, :, 2 * NLOC:KVCOLS].rearrange(
                            "r part (j h c) -> part r j h c", j=JT, h=4
                        ),
                    )

                    for p in gpairs:
                        accs = []
                        for hh in range(2):
                            accs.append(
                                acc_ps.tile([128, NLOC], f32, tag="acc",
                                            name=f"acc_{p}_{hh}")
                            )
                        for lt in range(LTILES):
                            sc = sc_ps.tile([128, 2, NLOC], f32, tag="sc")
                            for hh in range(2):
                                nc.tensor.matmul(
                                    sc[:, hh, :],
                                    kt_g[hh * 64:(hh + 1) * 64, p - 2 * g,
                                         lt * 128:(lt + 1) * 128],
                                    qt_t[hh * 64:(hh + 1) * 64, p, :],
                                    start=True,
                                    stop=True,
                                    tile_position=(hh * 64, 0),
                                )
                            ex = expp.tile([128, 2, NLOC], bf16, tag="exp")
                            nc.scalar.activation(
                                ex[:], sc[:],
                                mybir.ActivationFunctionType.Exp,
                                bias=zbias[:],
                            )
                            for hh in range(2):
                                nc.tensor.matmul(
                                    accs[hh][0:DH + 1, :],
                                    v_g[:, lt, 2 * (p - 2 * g) + hh, :],
                                    ex[:, hh, :],
                                    start=(lt == 0),
                                    stop=(lt == LTILES - 1),
                                )
                        for hh in range(2):
                            acc = accs[hh]
                            rs = small.tile([1, NLOC], f32, tag="recip")
                            nc.vector.reciprocal(rs[:], acc[DH:DH + 1, :])
                            rsb = small.tile([1, NLOC], bf16, tag="recipb")
                            nc.vector.tensor_scalar_mul(
                                rsb[:], rs[:], INV_SQRT_D
                            )
                            nc.tensor.matmul(
                                acc[64:128, :],
                                ones_row[:],
                                rsb[:],
                                start=True,
                                stop=True,
                                tile_position=(0, 64),
                            )
                            bcast_s = small.tile([64, NLOC], f32,
                                                 tag="bcast")
                            nc.vector.tensor_copy(bcast_s[:], acc[64:128, :])
                            att = attn_t[p][hh * 64:(hh + 1) * 64, :]
                            nc.vector.tensor_tensor(
                                att, acc[0:DH, :], bcast_s[:], mult
                            )
                            nc.vector.tensor_scalar_add(
                                att, att,
                                bias_t[hh * 64:hh * 64 + 64, 12 + p:13 + p],
                            )

            # ---- o-proj, natural layout: out[q, o] ----
            with tc.tile_pool(name="o_ps", bufs=2,
                              space=bass.MemorySpace.PSUM) as o_ps:
                for qt in range(QTILES):
                    fo = small.tile([128, D], f32, tag="final")
                    for oh in range(2):
                        osl = slice(oh * 384, (oh + 1) * 384)
                        ps = o_ps.tile([128, 384], f32, tag="op")
                        for dt in range(ITILES):
                            nc.tensor.matmul(
                                ps[:],
                                attn_t[dt][:, qt * 128:(qt + 1) * 128],
                                wo_t[dt][:, osl],
                                start=(dt == 0),
                                stop=False,
                            )
                        nc.tensor.matmul(
                            ps[:], ones_q[:], bo_bf[:, osl],
                            start=False, stop=True,
                        )
                        nc.vector.tensor_copy(fo[:, osl], ps[:])
                    nc.sync.dma_start(
                        out.rearrange("(qt p) f -> p qt f", p=128)[:, qt, :],
                        fo[:],
                    )

    nc.compile()
    return nc


def _prep_inputs(x, Wqkv, bqkv, Wo, bo):
    x2 = np.asarray(x, dtype=np.float32).reshape(N, D)
    Wqkv = np.asarray(Wqkv, dtype=np.float32)
    bqkv = np.asarray(bqkv, dtype=np.float32)
    Wo = np.asarray(Wo, dtype=np.float32)
    bo = np.asarray(bo, dtype=np.float32)

    h_idx = np.arange(H).repeat(DH)
    d_idx = np.tile(np.arange(DH), H)
    perm = h_idx * (3 * DH) + d_idx * 3
    biasp = np.ascontiguousarray(np.concatenate(
        [
            bqkv[perm + 0].reshape(ITILES, 128).T,
            bqkv[perm + 1].reshape(ITILES, 128).T,
            (bqkv[perm + 2] * INV_SQRT_D).reshape(ITILES, 128).T,
        ],
        axis=1,
    ).astype(np.float32))
    bo_row = np.ascontiguousarray(bo.reshape(1, D))

    in_maps = []
    for c in range(NCORES):
        in_maps.append({
            "xo": x2[c * NLOC:(c + 1) * NLOC],
            "wqkv_s": Wqkv[c * WSH:(c + 1) * WSH],
            "wo_s": Wo[c * WSH:(c + 1) * WSH],
            "biasp": biasp,
            "bo_row": bo_row,
        })
    return in_maps


def kernel(x, Wqkv, bqkv, Wo, bo, _trace=False, _trace_cores=None):
    from concourse.bass_utils import run_bass_kernel_spmd

    if "nc" not in _cache:
        _cache["nc"] = _build_program()
    nc = _cache["nc"]

    in_maps = _prep_inputs(x, Wqkv, bqkv, Wo, bo)
    res = run_bass_kernel_spmd(
        nc, in_maps, list(range(NCORES)), trace=_trace,
        trace_cores=_trace_cores,
    )
    _cache["last_results"] = res
    out = np.concatenate(
        [res.results[c]["out"] for c in range(NCORES)], axis=0
    )
    return np.ascontiguousarray(out.reshape(1, N, D).astype(np.float32))
